# revision 71
# baseline (speedup 1.0000x reference)
"""Trainium2 Bass kernel: single-head causal attention (v2).

Problem: x[4,4096,128]; Q/K/V linear projections (W [in,out] layout, +bias);
scores = QK^T/sqrt(128) with causal mask; softmax; out = P @ V.

Sharding (8 cores = 4 batches x 2), SPMD (all cores run one program):
  core (b, h):
    triangle: queries q in [2048h, 2048h+2048) of batch b attending causally
        to kv rows in the same range.
    rectangle: queries q in [2048, 4096) attending to kv [1024h, 1024h+1024)
        (fully valid, no mask).
  Union over both cores of a batch covers the full causal set exactly once.

Softmax without max subtraction but with a constant shift: every score gets
-4.0 before exp (softmax shift-invariance keeps O/l exact), which brings
P = exp(s-4) into fp8 e4m3 range (max ~6 << 240). The cross-core merge stays
linear: host sums unnormalized o and denominators l, then divides.

Numerics by stage (validated to 8.6e-3 rel err vs the fp32 reference,
gate 2e-2):
  - x and W DMA'd in bf16; projections are bf16 matmuls (1 cycle/row always,
    no fp32r narrow-output penalty), PSUM fp32.
  - QT/KT stored f32r; score matmuls (ST) run fp32r (full rate >= 256 wide).
  - fp8 e4m3 P~T + V everywhere except chunk 0's diagonal pairs: AV and the
    denominator matmul run in DoubleRow perf mode (two 128-kv tiles per
    pass, 0.5 cycles/row = 4x the fp32r rate).
  - Chunk 0's diagonal pairs stay f32r end to end: its short softmax rows
    (1..512 kv) have no error averaging, and fp8 there fails the gate
    (5.8e-2). Chunks 1-3 diagonals are fp8 with a uniform per-pair lo
    (0 for tiles m<2, 256 for m>=2) so the DoubleRow halves are uniform.
  - bk drops out of softmax; bq (pre-scaled) added during the Q PSUM->SBUF
    copy; bv added by the host after normalization.

Schedule: the exp stream on the scalar engine (~36.5us busy) is the
critical resource, with PE at ~31us underneath it. Projections are
software-pipelined INTO the attention chunk stream (one projection matmul
thunk per attention unit, emitted AFTER the unit's ST/exp/AV) so ACT
starts exp'ing ~6us in and rarely starves. Chunk 0's prologue projections
go to four parallel PSUM banks (borrowing the idle po/pl banks) with its
q/k copies on the still-idle ACT. Mask adds are bf16 identity-matmuls on
the PE. All other PSUM->SBUF copies are on DVE (GPSIMD cannot touch PSUM;
it does the SBUF->SBUF fp8 V conversion). Skew-1 software pipeline: AV+l
of unit u are emitted after ST/exp of unit u+1 (deeper skew backs up the
po/pl accumulator recycling and measures worse). PSUM banks: st 2x2 +
proj 2x1 + po 1 + pl 1 = 8.

Device layouts (per core):
  xin  [128,5120] bf16  x^T cols: 0:4096 = (tri|rect_q), 4096:5120 = rect_kv
  cb16 [128,768]  bf16  wq|wk|wv|msk(256)|ident
  cf32 [128,132]  f32r  bq' | -4.0 | pad | ones32[128] (diag l stationary)
  ones8 [128,256] fp8   all-ones DoubleRow l stationary
  QT [128,4096] f32r; KT [128,3072] f32r
  vsb32 [128,512] f32r (V tiles 0-3, chunk-0 diag); vsb8 [128,3072] fp8
  ST computed transposed: ST[k,q] = K Q^T in PSUM; exp(ST-4) -> P~T in
  fp8 (or f32r for chunk-0 diag); AV: oT[e,q] += V^T-mm-P~T;
  l[q] += ones-mm-P~T (PE is the only partition reducer; the l stationary
  is 128 columns of ones because the ISA rejects narrow DoubleRow
  ldweights - every PSUM row of pl holds a copy of l, row 0 is used).
Outputs: oT [128,4096] f32 (transposed, unnormalized), lv [1,4096]
(denominators, chunk-major). Host transposes, merges, normalizes, adds bv.

TimelineSim cost: 49.8us/core vs the 73.8us baseline this replaced.
"""

import math
import sys

import numpy as np

sys.path.insert(0, "/opt/trn_rl_repo")

import concourse.bass as bass  # noqa: E402
import concourse.mybir as mybir  # noqa: E402
from concourse.tile import TileContext  # noqa: E402

B, T, D = 4, 4096, 128
HALF = T // 2          # 2048 queries per triangle
NCHUNK = 8             # 8 chunks of 512 query slots (4 tri + 4 rect)
CHUNK = 512
KV_TILES = 24          # 16 tri + 8 rect kv tiles of 128 rows
NEG = -99840.0         # additive mask value, exactly representable in bf16
SHIFT = 4.0            # score shift: exp(s - 4) keeps P in fp8 range

F32 = mybir.dt.float32
F32R = mybir.dt.float32r
BF16 = mybir.dt.bfloat16
F8 = mybir.dt.float8e4
DR = mybir.MatmulPerfMode.DoubleRow


def round_f32r(a):
    """Exact fp32 -> fp32r rounding (RNE to 11 mantissa bits)."""
    u = np.ascontiguousarray(a, np.float32).view(np.uint32)
    add = np.uint32(0x7FF) + ((u >> np.uint32(12)) & np.uint32(1))
    return ((u + add) & np.uint32(0xFFFFF000)).view(np.float32)


def build_nc(legalize=True):
    nc = bass.Bass()

    xin_d = nc.declare_dram_parameter("xin", [D, 5120], BF16, isOutput=False)
    cb16_d = nc.declare_dram_parameter("cb16", [D, 768], BF16, isOutput=False)
    cf32_d = nc.declare_dram_parameter("cf32", [D, 132], F32R, isOutput=False)
    ones8_d = nc.declare_dram_parameter("ones8", [D, 2 * D], F8, isOutput=False)

    ot_d = nc.declare_dram_parameter("oT", [D, T], F32, isOutput=True)
    lv_d = nc.declare_dram_parameter("lv", [1, T], F32, isOutput=True)

    with TileContext(nc) as tc:
        with (
            tc.tile_pool(name="sb", bufs=1) as sb,
            tc.tile_pool(name="stp", bufs=1, space="PSUM") as stp,
            tc.tile_pool(name="pp", bufs=2, space="PSUM") as pp,
            tc.tile_pool(name="op", bufs=1, space="PSUM") as op,
            tc.tile_pool(name="lp", bufs=1, space="PSUM") as lp,
            tc.tile_pool(name="osb", bufs=4) as osb,
        ):
            # ---- input DMAs, critical-path order: stage 0 (weights + xin
            # cols 0:512 + consts) lands first so PE starts ~2.6us in ----
            cb16 = sb.tile([D, 768], BF16)
            nc.sync.dma_start(out=cb16, in_=cb16_d[:, :])
            xin = sb.tile([D, 5120], BF16)
            nc.sync.dma_start(out=xin[:, 0:512], in_=xin_d[:, 0:512])
            cf32 = sb.tile([D, 132], F32R)
            nc.sync.dma_start(out=cf32, in_=cf32_d[:, :])
            nc.sync.dma_start(out=xin[:, 512:1024], in_=xin_d[:, 512:1024])
            nc.sync.dma_start(out=xin[:, 1024:2048], in_=xin_d[:, 1024:2048])
            ones8 = sb.tile([D, 2 * D], F8)
            nc.sync.dma_start(out=ones8, in_=ones8_d[:, :])
            nc.sync.dma_start(out=xin[:, 4096:5120], in_=xin_d[:, 4096:5120])
            nc.sync.dma_start(out=xin[:, 2048:3072], in_=xin_d[:, 2048:3072])
            nc.sync.dma_start(out=xin[:, 3072:4096], in_=xin_d[:, 3072:4096])

            wq = cb16[:, 0:128]
            wk = cb16[:, 128:256]
            wv = cb16[:, 256:384]
            msk = cb16[:, 384:640]     # [0:128) all NEG | [128:256) staircase
            ident = cb16[:, 640:768]
            bq = cf32[:, 0:1].bitcast(F32)
            sh4 = cf32[:, 1:2].bitcast(F32)  # -4.0 exp bias
            # l-matmul stationaries: 128-wide all-ones (the ISA rejects
            # narrow DoubleRow ldweights; a full-width stationary costs the
            # same moving columns and makes every PSUM row a copy of l)
            ones32 = cf32[:, 4:132]

            qt = sb.tile([D, T], F32R)
            kt = sb.tile([D, KV_TILES * 128], F32R)
            vsb32 = sb.tile([D, CHUNK], F32R)   # V tiles 0-3 (chunk-0 diag)
            vsb8 = sb.tile([D, KV_TILES * 128], F8)
            lvs = sb.tile([1, T], F32)

            # ---- projection stages (pipelined into the attention stream).
            # Copies alternate DVE / GPSIMD so ACT stays exp-only. ----
            def xcol(t):  # xin column of kv tile t
                return t * 128 if t < 16 else 4096 + (t - 16) * 128

            def v_stage(g, pool=None):   # V tiles 4g..4g+3 -> [kvrow, e]
                ps = (pool or pp).tile(
                    [D, CHUNK], F32,
                    **({"tag": "po", "name": "po"} if pool is not None
                       else {"tag": "pp", "name": f"psv{g}"}))
                for jj in range(4):
                    t = 4 * g + jj
                    nc.tensor.matmul(
                        ps[:, jj * 128:(jj + 1) * 128],
                        xin[:, xcol(t):xcol(t) + 128], wv,
                        start=True, stop=True, skip_group_check=True,
                    )
                # GPSIMD cannot touch PSUM: DVE drains the bank, GPSIMD does
                # the off-critical-path SBUF->SBUF fp8 conversion.
                sl = slice(g * CHUNK, (g + 1) * CHUNK)
                if g == 0:
                    nc.vector.tensor_copy(vsb32, ps)
                    nc.gpsimd.tensor_copy(vsb8[:, sl], vsb32)
                else:
                    nc.vector.tensor_copy(vsb8[:, sl], ps)

            def k_stage(j, pool=None, bname=None, split=False):  # K^T chunk j
                ps = (pool or pp).tile(
                    [D, CHUNK], F32,
                    **({"tag": bname, "name": bname} if pool is not None
                       else {"tag": "pp", "name": f"psk{j}"}))
                src = xin[:, j * CHUNK:(j + 1) * CHUNK] if j < 4 else \
                    xin[:, 4096 + (j - 4) * CHUNK:4096 + (j - 3) * CHUNK]
                nc.tensor.matmul(ps, wk, src,
                                 start=True, stop=True, skip_group_check=True)
                sl = slice(j * CHUNK, (j + 1) * CHUNK)
                if split:
                    # chunk-0 critical path: two halves on the still-idle
                    # ACT, so pair-1's ST (kv tiles 0-1) gates on the first
                    # half only
                    h = CHUNK // 2
                    nc.scalar.copy(kt[:, sl][:, 0:h], ps[:, 0:h])
                    nc.scalar.copy(kt[:, sl][:, h:], ps[:, h:])
                else:
                    nc.vector.tensor_copy(kt[:, sl], ps)

            def q_stage(c):          # Q^T chunk c (scaled, biased)
                ps = pp.tile([D, CHUNK], F32, tag="pp", name=f"psq{c}")
                nc.tensor.matmul(ps, wq, xin[:, c * CHUNK:(c + 1) * CHUNK],
                                 start=True, stop=True, skip_group_check=True)
                sl = slice(c * CHUNK, (c + 1) * CHUNK)
                if c == 0:
                    # chunk-0 critical path: bias-copy on the idle ACT
                    nc.scalar.activation(
                        qt[:, sl], ps,
                        mybir.ActivationFunctionType.Identity, bias=bq)
                else:
                    nc.vector.tensor_scalar_add(qt[:, sl], ps, bq)

            # two persistent score buffers, manually rotated. The merged
            # diag exp reads a small inter-tile gap; only chunk 0's first
            # use of each buffer sees it uninitialized (stale-but-finite
            # afterwards, never consumed) - zero exactly those windows.
            sts = [stp.tile([D, 2 * CHUNK], F32, name=f"st{i}")
                   for i in range(2)]
            nc.vector.memset(sts[0][:, CHUNK:CHUNK + 128], 0.0)
            nc.vector.memset(sts[1][:, CHUNK:CHUNK + 256], 0.0)
            nst = [0]

            # prologue: only what chunk 0 needs, on FOUR parallel PSUM
            # banks (op/lp are idle until the first AV, so the v0/k0/k1
            # projections borrow them -> no copy->matmul WAR chain).
            # chunk-0's q/k copies ride the still-idle ACT.
            q_stage(0)
            k_stage(0, split=True)
            v_stage(0, pool=op)
            k_stage(1, pool=lp, bname="pl")
            thunkq = [lambda: q_stage(1), lambda: v_stage(1)]
            stage_thunks = {
                2: [lambda: v_stage(2), lambda: k_stage(2), lambda: q_stage(2)],
                3: [lambda: v_stage(3), lambda: k_stage(3), lambda: q_stage(3)],
                4: [lambda: v_stage(4), lambda: v_stage(5), lambda: k_stage(4),
                    lambda: k_stage(5), lambda: q_stage(4)],
                5: [lambda: q_stage(5)],
                6: [lambda: q_stage(6)],
                7: [lambda: q_stage(7)],
            }

            # ---- attention: 8 chunks; units are kv-tile pairs.
            # Tri chunks: 2 diagonal (masked) pairs first, then full pairs
            # (fp8 DoubleRow) descending. Only chunk 0's diagonals (the
            # short softmax rows, no error averaging) stay f32r; chunks
            # 1-3 diagonals run fp8 DoubleRow with a uniform per-pair lo.
            # Skew-1 software pipeline: AV+l of unit u are emitted after
            # ST/exp of unit u+1. Kinds: 'f32' chunk-0 diag, 'f8d' fp8
            # diag (masked), 'f8' full. ----
            units = []
            for c in range(NCHUNK):
                if c == 0:
                    pairs = [((0, 1), (0, 128), "f32"),
                             ((2, 3), (256, 256), "f32")]
                elif c < 4:
                    pairs = [((4 * c, 4 * c + 1), (0, 0), "f8d"),
                             ((4 * c + 2, 4 * c + 3), (256, 256), "f8d")]
                    for t0 in range(4 * c - 2, -1, -2):
                        pairs.append(((t0, t0 + 1), (0, 0), "f8"))
                else:
                    pairs = [((16 + 2 * i, 17 + 2 * i), (0, 0), "f8")
                             for i in range(4)]
                for pi, (pr, los, kind) in enumerate(pairs):
                    units.append((c, pr, los, kind, pi == 0,
                                  pi == len(pairs) - 1))

            pts8 = [sb.tile([D, 2 * CHUNK], F8, name=f"pt8_{i}")
                    for i in range(4)]
            pts32 = [sb.tile([D, 2 * CHUNK], F32R, name=f"pt32_{i}")
                     for i in range(2)]
            n8 = [0]
            n32 = [0]
            acc = {}                # chunk -> (po, pl)
            pend_q = []             # skew-2: AV(u) emitted after exp(u+2)
            epiq = []

            def emit_epilogue():
                c, po, pl = epiq.pop(0)
                qsl = slice(c * CHUNK, (c + 1) * CHUNK)
                ob = osb.tile([D, CHUNK], F32, tag="ob", name="ob")
                if c == NCHUNK - 1:
                    # tail: lv copy on the now-idle ACT, parallel to the
                    # ob copy on DVE
                    nc.scalar.copy(lvs[:, qsl], pl[0:1, :])
                else:
                    # lv BEFORE ob: lp has a single buffer, so releasing pl
                    # first lets the next chunk's first l-matmul proceed
                    nc.vector.tensor_copy(lvs[:, qsl], pl[0:1, :])
                nc.vector.tensor_copy(ob, po)
                nc.sync.dma_start(out=ot_d[:, qsl], in_=ob)
                if c == NCHUNK - 2:
                    # flush chunks 0..6 denominators off the tail early
                    nc.sync.dma_start(out=lv_d[:, 0:(NCHUNK - 1) * CHUNK],
                                      in_=lvs[:, 0:(NCHUNK - 1) * CHUNK])
                elif c == NCHUNK - 1:
                    nc.sync.dma_start(out=lv_d[:, qsl], in_=lvs[:, qsl])

            def emit_av(pend):
                c, pr, los, kind, is_first, is_last, pt = pend
                if c not in acc:
                    acc[c] = (
                        op.tile([D, CHUNK], F32, tag="po", name="po"),
                        lp.tile([D, CHUNK], F32, tag="pl", name="pl"),
                    )
                po, pl = acc[c]
                if kind != "f32":
                    lo = los[0]       # uniform per-pair lo for fp8 kinds
                    v3 = vsb8[:, pr[0] * 128:(pr[0] + 2) * 128].rearrange(
                        "p (k f) -> p k f", k=2)
                    p3 = pt.rearrange("p (k f) -> p k f", k=2)[:, :, lo:]
                    o3 = ones8.rearrange("p (k f) -> p k f", k=2)
                    nc.tensor.matmul(po[:, lo:], v3, p3, start=is_first,
                                     stop=is_last, perf_mode=DR,
                                     skip_group_check=True)
                    nc.tensor.matmul(pl[:, lo:], o3, p3, start=is_first,
                                     stop=is_last, perf_mode=DR,
                                     skip_group_check=True)
                else:
                    for i, t in enumerate(pr):
                        lo = los[i]
                        ptc = pt[:, i * CHUNK + lo:(i + 1) * CHUNK]
                        st_f = is_first and i == 0
                        sp_f = is_last and i == 1
                        nc.tensor.matmul(
                            po[:, lo:], vsb32[:, t * 128:(t + 1) * 128], ptc,
                            start=st_f, stop=sp_f, skip_group_check=True)
                        nc.tensor.matmul(
                            pl[:, lo:], ones32, ptc,
                            start=st_f, stop=sp_f, skip_group_check=True)
                if is_last:
                    epiq.append((c, po, pl))
                    del acc[c]

            for c, pr, los, kind, is_first, is_last in units:
                if is_first and (c + 2) in stage_thunks:
                    thunkq.extend(stage_thunks[c + 2])
                if epiq:
                    emit_epilogue()
                st = sts[nst[0] % 2]
                nst[0] += 1
                for i, t in enumerate(pr):
                    lo = los[i]
                    nc.tensor.matmul(
                        st[:, i * CHUNK + lo:(i + 1) * CHUNK],
                        kt[:, t * 128:(t + 1) * 128],
                        qt[:, c * CHUNK + lo:(c + 1) * CHUNK],
                        start=True, stop=True, skip_group_check=True,
                    )
                    if kind != "f8":
                        # causal mask band over [lo, 128(m+1)): width w
                        # staircase tail, all-NEG before it. msk stores
                        # [allNEG(128) | staircase(128)]; slice the last
                        # w columns.
                        m = t - 4 * c
                        w = 128 * (m + 1) - lo
                        nc.tensor.matmul(
                            st[:, i * CHUNK + lo:i * CHUNK + lo + w],
                            ident, msk[:, 256 - w:256],
                            start=False, stop=True, skip_group_check=True)
                if kind == "f32":
                    pt = pts32[n32[0] % 2]
                    n32[0] += 1
                else:
                    pt = pts8[n8[0] % 4]
                    n8[0] += 1
                # one exp per pair; the inter-tile gap region (columns
                # [CHUNK, CHUNK+los[1]) when los[1] > los[0]) holds stale
                # PSUM, is exp'd harmlessly, and is never read downstream.
                nc.scalar.activation(
                    pt[:, los[0]:], st[:, los[0]:],
                    mybir.ActivationFunctionType.Exp, bias=sh4)
                # skew-1 software pipeline: AV+l of unit u are emitted after
                # ST/exp of unit u+1, so the PE never waits on ACT (deeper
                # skew measures worse: the deferred AVs back up the
                # epilogue/accumulator recycling chain)
                pend_q.append((c, pr, los, kind, is_first, is_last, pt))
                if len(pend_q) > 1:
                    emit_av(pend_q.pop(0))
                # projection thunks AFTER the unit's critical ST/exp/AV
                # emissions, ONE per unit: the exp stream gets PE priority
                # and the projection matmuls spread into the slack (the
                # 34-unit stream drains all 14 thunks well before their
                # consumer chunks)
                if thunkq:
                    thunkq.pop(0)()
            for p in pend_q:
                emit_av(p)
            while epiq:
                emit_epilogue()

    if legalize:
        _legalize_multiwaits(nc)
    nc.finalize()
    return nc


def _legalize_multiwaits(nc):
    """Hardware instruction structs in this walrus build accept at most ONE
    sync wait. Move all but the last wait onto single-wait same-engine NoOps
    inserted right before the instruction (engines execute in order)."""
    for fn in nc.m.functions:
        for blk in fn.blocks:
            insts = blk.instructions
            out = []
            for inst in insts:
                si = inst.sync_info
                if si is not None and si.on_wait and len(si.on_wait) >= 2:
                    waits = list(si.on_wait)
                    for w in waits[:-1]:
                        out.append(mybir.InstNoOp(
                            name=nc.get_next_instruction_name(),
                            engine=inst.engine,
                            bass_nofuse=True,
                            sync_info=mybir.SyncInfo(
                                on_wait=[w], on_update=[]),
                        ))
                    inst.sync_info = mybir.SyncInfo(
                        on_wait=[waits[-1]],
                        on_update=list(si.on_update or []))
                out.append(inst)
            insts[:] = out


_NC_CACHE = {}


def get_nc(legalize=True):
    key = ("nc", legalize)
    if key not in _NC_CACHE:
        _NC_CACHE[key] = build_nc(legalize)
    return _NC_CACHE[key]


def make_core_inputs(x, Wq, bq, Wk, bk, Wv, bv):
    """Per-core input maps (host-side sharding). bk dropped (softmax
    invariance); bv applied on the host."""
    import ml_dtypes

    s = 1.0 / math.sqrt(D)
    wq16 = (np.asarray(Wq, np.float32) * s).astype(ml_dtypes.bfloat16)
    wk16 = np.asarray(Wk, np.float32).astype(ml_dtypes.bfloat16)
    wv16 = np.asarray(Wv, np.float32).astype(ml_dtypes.bfloat16)

    # msk: cols [0:128) all NEG; [128:256) staircase 0 if j >= k else NEG
    kk = np.arange(128)[:, None]
    jj = np.arange(128)[None, :]
    stair = np.where(jj >= kk, 0.0, NEG).astype(np.float32)
    mskf = np.concatenate([np.full((D, 128), NEG, np.float32), stair], axis=1)
    identf = np.eye(D, dtype=np.float32)
    cb16 = np.concatenate(
        [wq16, wk16, wv16,
         mskf.astype(ml_dtypes.bfloat16), identf.astype(ml_dtypes.bfloat16)],
        axis=1)  # [D, 768] bf16

    cf32 = np.zeros((D, 132), np.float32)
    cf32[:, 0] = np.asarray(bq, np.float32) * s
    cf32[:, 1] = -SHIFT
    cf32[:, 4:132] = 1.0     # f32r all-ones stationary for diag l-matmuls
    ones8 = np.ones((D, 2 * D), ml_dtypes.float8_e4m3)

    x = np.asarray(x, dtype=np.float32)
    in_maps = []
    for core in range(8):
        b, h = core // 2, core % 2
        xb = x[b]
        tri = xb[h * HALF:(h + 1) * HALF]          # [2048, 128]
        rect_q = xb[HALF:]                         # [2048, 128]
        rect_kv = xb[h * 1024:(h + 1) * 1024]      # [1024, 128]
        xin = np.ascontiguousarray(
            np.concatenate([tri, rect_q, rect_kv], axis=0).T
        ).astype(ml_dtypes.bfloat16)               # [128, 5120]
        in_maps.append({"xin": xin, "cb16": cb16, "cf32": cf32,
                        "ones8": ones8})
    return in_maps


def merge_outputs(results, bv):
    """Gather per-core (oT, lv) into the full [B, T, D] output. The -4
    score shift scales o and l identically, so it cancels in o/l."""
    bv = np.asarray(bv, dtype=np.float32)
    out = np.empty((B, T, D), np.float32)
    for b in range(B):
        lo, hi = results[2 * b], results[2 * b + 1]
        lo_lv = np.asarray(lo["lv"]).reshape(NCHUNK, CHUNK)
        hi_lv = np.asarray(hi["lv"]).reshape(NCHUNK, CHUNK)
        O = np.zeros((T, D), np.float64)
        L = np.zeros(T, np.float64)
        O[:HALF] += lo["oT"][:, :HALF].T
        L[:HALF] += lo_lv[0:4].ravel()
        O[HALF:] += hi["oT"][:, :HALF].T
        L[HALF:] += hi_lv[0:4].ravel()
        O[HALF:] += lo["oT"][:, HALF:].T
        L[HALF:] += lo_lv[4:8].ravel()
        O[HALF:] += hi["oT"][:, HALF:].T
        L[HALF:] += hi_lv[4:8].ravel()
        out[b] = (O / L[:, None]).astype(np.float32) + bv
    return out


def run_per_core(nc, in_maps, threads=True):
    """Run the same single-core program on each NeuronCore with its own
    inputs (per-core dispatch; the cores share no collectives)."""
    import jax
    from concourse import bass2jax

    devices = jax.devices()[:len(in_maps)]

    def one(i):
        with jax.default_device(devices[i]):
            return bass2jax.run_bass_via_pjrt(nc, [in_maps[i]], n_cores=1)[0]

    if threads:
        from concurrent.futures import ThreadPoolExecutor
        first = one(0)
        with ThreadPoolExecutor(max_workers=7) as ex:
            rest = list(ex.map(one, range(1, len(in_maps))))
        return [first] + rest
    return [one(i) for i in range(len(in_maps))]


def kernel(x, Wq, bq, Wk, bk, Wv, bv, _trace=False):
    from concourse.bass_utils import axon_active, run_bass_kernel_spmd

    nc = get_nc()
    in_maps = make_core_inputs(x, Wq, bq, Wk, bk, Wv, bv)
    if axon_active():
        results = run_per_core(nc, in_maps)
    else:
        res = run_bass_kernel_spmd(nc, in_maps, list(range(8)), trace=_trace)
        kernel.last_result = res
        results = res.results
    out = merge_outputs(results, bv)
    return out
